# revision 1
# baseline (speedup 1.0000x reference)
"""Trainium2 Bass kernel for nn_DilatedAttention (dynamic per-image 3x3
depthwise filter + affine epilogue), data-parallel over batch on 8 cores.

Math per image (one core):
  pooled[c] = mean_hw(x)                              (64,)
  lf = tanh(BN(pooled @ conv_w.T))                    (72,) = (G=8, k2=9)
  low[c,h,w] = sum_t lf[g(c),t] * x[c, h+di, w+dj]    3x3 reflect-pad conv
  out = A[c]*low + B[c]*x + const[c]
    A = lamb_l*(1+inside_all), B = 1+lamb_h, const = -inside_all*lamb_l*pooled

Device layout: partition p = half*64 + c (two 128-row halves of the image),
free dims = [130 rows x 258 cols] with reflect halo rows/cols so every
3x3 tap is a pure free-dim offset.  The 9 taps run as 9 PE matmuls with
diagonal (per-partition-scale) stationary matrices in float32r,
accumulating in PSUM; B*x folds into the center-tap diagonal; const[c]
folds into the ScalarE PSUM->SBUF evacuation bias.
"""

import os
import sys

import numpy as np

for _p in ("/opt/trn_rl_repo",):
    if _p not in sys.path:
        sys.path.insert(0, _p)

import concourse.bass as bass
import concourse.bacc as bacc
import concourse.mybir as mybir
import concourse.tile as tile
from concourse.bass_utils import run_bass_kernel_spmd

F32 = mybir.dt.float32
F32R = mybir.dt.float32 if os.environ.get("KERNEL_MM_F32") else mybir.dt.float32r
AF = mybir.ActivationFunctionType
ALU = mybir.AluOpType

C, H, W = 64, 256, 256
NCORES = 8
K2 = 9
ROWS_PER_TILE = 2          # image rows per PSUM tile (2*256 = 512 = one bank)
NTILES = 128 // ROWS_PER_TILE
N_DVE_PAIRS_PER8 = 1      # of every 4 tile-pairs, this many go to the DVE pipeline

LAST_RESULT = {}


def _install_ntff_hook():
    """Register the axon NTFF profile hook (the image's antenv lacks
    axon_hooks; build it from trn_agent_boot's ctypes shim)."""
    import types

    try:
        from antenv.axon_hooks import get_axon_ntff_profile_hook  # noqa: F401
        return
    except ImportError:
        pass
    mod = types.ModuleType("antenv.axon_hooks")
    _h = [None]
    mod.set_axon_ntff_profile_hook = lambda hook: _h.__setitem__(0, hook)
    mod.get_axon_ntff_profile_hook = lambda: _h[0]
    sys.modules["antenv.axon_hooks"] = mod
    import antenv

    antenv.axon_hooks = mod
    try:
        from trn_agent_boot.trn_boot import _ntff_profile_via_ctypes

        mod.set_axon_ntff_profile_hook(
            _ntff_profile_via_ctypes("/opt/axon/libaxon_pjrt.so")
        )
    except Exception as e:  # hook stays None; tracing degrades gracefully
        print("ntff hook install failed:", e)


def _build_program():
    nc = bacc.Bacc("TRN2", target_bir_lowering=False, debug=False)

    x_d = nc.declare_dram_parameter("x", [C, H, W], F32R, isOutput=False)
    out_d = nc.declare_dram_parameter("out", [C, H, W], F32, isOutput=True)
    cwT_d = nc.declare_dram_parameter("cwT", [64, 72], F32, isOutput=False)
    bns_d = nc.declare_dram_parameter("bns", [72, 1], F32, isOutput=False)
    bnb_d = nc.declare_dram_parameter("bnb", [72, 1], F32, isOutput=False)
    g72_d = nc.declare_dram_parameter("g72", [72, 128], F32, isOutput=False)
    mask9_d = nc.declare_dram_parameter("mask9", [72, K2], F32, isOutput=False)
    ppool_d = nc.declare_dram_parameter("ppool", [128, 128], F32, isOutput=False)
    i128_d = nc.declare_dram_parameter("i128", [128, 128], F32R, isOutput=False)
    avec_d = nc.declare_dram_parameter("avec", [128, 1], F32, isOutput=False)
    bvec_d = nc.declare_dram_parameter("bvec", [128, 1], F32, isOutput=False)
    clvec_d = nc.declare_dram_parameter("clvec", [128, 1], F32, isOutput=False)

    with tile.TileContext(nc) as tc:
        with (
            tc.tile_pool(name="xbuf", bufs=1) as xp,
            tc.tile_pool(name="consts", bufs=1) as cp,
            tc.tile_pool(name="diag", bufs=1) as dp,
            tc.tile_pool(name="psum", bufs=4, space=bass.MemorySpace.PSUM) as pp,
            tc.tile_pool(name="stage", bufs=3) as sp,
            tc.tile_pool(name="spsum", bufs=1, space=bass.MemorySpace.PSUM) as pps,
        ):
            # ---- load x into SBUF (contiguous 256-wide rows + halo rows) ----
            # Top half p<64: layout row r holds HBM row r-1; bottom half
            # p>=64: layout r holds HBM row 127+r.  Each half loads through
            # its own HW-DGE ring (SP / ACT) so the two 64-partition streams
            # run concurrently.  6 chunks + 2-row gaps per half let pooling
            # overlap the load.
            x_sb = xp.tile([128, 130, 256], F32R)
            segs = [(1, 22), (24, 45), (47, 68), (70, 91), (93, 114), (116, 129)]
            gaps = [(22, 24), (45, 47), (68, 70), (91, 93), (114, 116)]
            for a, b in segs + gaps:
                nc.sync.dma_start(out=x_sb[0:64, a:b, :], in_=x_d[:, a - 1:b - 1, :])
                nc.sync.dma_start(out=x_sb[64:128, a:b, :],
                                  in_=x_d[:, 127 + a:127 + b, :])
            # top halo-bottom (HBM row 128) and bottom halo-top (HBM row 127)
            nc.sync.dma_start(out=x_sb[0:64, 129:130, :], in_=x_d[:, 128:129, :])
            nc.sync.dma_start(out=x_sb[64:128, 0:1, :], in_=x_d[:, 127:128, :])
            # reflect halo rows via on-chip copies
            nc.vector.tensor_copy(x_sb[0:64, 0:1, :], x_sb[0:64, 2:3, :])
            nc.vector.tensor_copy(x_sb[64:128, 129:130, :],
                                  x_sb[64:128, 127:128, :])

            # ---- small constants ----
            cwT = cp.tile([64, 72], F32, tag="cwT")
            bns = cp.tile([72, 1], F32, tag="bns")
            bnb = cp.tile([72, 1], F32, tag="bnb")
            g72 = cp.tile([72, 128], F32, tag="g72")
            mask9 = cp.tile([72, K2], F32, tag="mask9")
            ppool = cp.tile([128, 128], F32, tag="ppool")
            i128 = cp.tile([128, 128], F32R, tag="i128")
            avec = cp.tile([128, 1], F32, tag="avec")
            bvec = cp.tile([128, 1], F32, tag="bvec")
            clvec = cp.tile([128, 1], F32, tag="clvec")
            for t, d in (
                (cwT, cwT_d), (bns, bns_d), (bnb, bnb_d), (g72, g72_d),
                (mask9, mask9_d), (ppool, ppool_d), (i128, i128_d),
                (avec, avec_d), (bvec, bvec_d), (clvec, clvec_d),
            ):
                nc.sync.dma_start(out=t[:], in_=d[:])

            # ---- pooling over layout rows 1..128, chunk-aligned w/ DMAs ----
            # Top-half chunks + bottom gaps: DVE tensor_reduce.  Bottom big
            # chunks: ScalarE in-place Copy with accum_out (f32r out keeps
            # the values bit-identical).
            NPS = len(segs) + len(gaps)
            pstat = cp.tile([128, NPS], F32, tag="pstat")
            for k, (a, b) in enumerate(segs + gaps):
                nc.vector.tensor_reduce(
                    out=pstat[0:64, k:k + 1],
                    in_=x_sb[0:64, a:b, :].bitcast(F32),
                    axis=mybir.AxisListType.XY,
                    op=ALU.add,
                )
            for k, (a, b) in enumerate(segs):
                nc.scalar.activation(
                    x_sb[64:128, a:b, :],
                    x_sb[64:128, a:b, :],
                    AF.Copy,
                    accum_out=pstat[64:128, k:k + 1],
                )
            for k, (a, b) in enumerate(gaps):
                nc.vector.tensor_reduce(
                    out=pstat[64:128, len(segs) + k:len(segs) + k + 1],
                    in_=x_sb[64:128, a:b, :].bitcast(F32),
                    axis=mybir.AxisListType.XY,
                    op=ALU.add,
                )
            stat = cp.tile([128, 1], F32, tag="stat")
            nc.vector.tensor_reduce(
                out=stat[:], in_=pstat[:], axis=mybir.AxisListType.X, op=ALU.add
            )

            # pooled[p] = (stat[p%64] + stat[64+p%64]) / 65536  (both halves)
            pooled_ps = pps.tile([128, 1], F32, tag="pooled_ps")
            nc.tensor.matmul(pooled_ps[:], ppool[:], stat[:])
            pooled = cp.tile([128, 1], F32, tag="pooled")
            nc.scalar.copy(pooled[:], pooled_ps[:])

            # const[p] = CL[p] * pooled[p]
            cvec = cp.tile([128, 1], F32, tag="cvec")
            nc.vector.tensor_scalar_mul(cvec[:], pooled[:], clvec[:])

            # lf = tanh(bns * (pooled @ conv_w.T) + bnb)   [72,1]
            lf_ps = pps.tile([72, 1], F32, tag="lf_ps")
            nc.tensor.matmul(lf_ps[:], cwT[:], pooled[0:64, :])
            lf = cp.tile([72, 1], F32, tag="lf")
            nc.scalar.activation(lf[:], lf_ps[:], AF.Tanh, bias=bnb[:], scale=bns[:])

            # W0[p,t] = lf[g(p)*9+t]:  lfmat = mask9 * lf ; W0 = g72.T @ lfmat
            lfmat = cp.tile([72, K2], F32, tag="lfmat")
            nc.vector.tensor_scalar_mul(lfmat[:], mask9[:], lf[:])
            w_ps = pps.tile([128, K2], F32, tag="w_ps")
            nc.tensor.matmul(w_ps[:], g72[:], lfmat[:])
            # W = A * W0 ; then center tap += B  (folds B*x into the conv)
            wmat = cp.tile([128, K2], F32, tag="wmat")
            nc.scalar.activation(wmat[:], w_ps[:], AF.Copy, scale=avec[:])
            nc.vector.tensor_scalar_add(wmat[:, 4:5], wmat[:, 4:5], bvec[:])

            # diagonal stationary matrices D_t = diag(W[:,t])
            diags = []
            for t in range(K2):
                d_t = dp.tile([128, 128], F32R, tag=f"d{t}")
                nc.vector.tensor_scalar_mul(d_t[:], i128[:], wmat[:, t:t + 1])
                diags.append(d_t)

            # ---- main loop ----
            # Tiles: 2 image rows x 256 cols x 128 partitions.  Horizontal
            # reflect is handled by splitting side taps into an N=255 body
            # (pure column shift within the row) + an N=1 edge fix.  Most
            # tiles: accumulating diag-matmuls on PE + ScalarE evacuation
            # (adds const); tiles with r%8 in {6,7}: DVE tensor_scalar/
            # scalar_tensor_tensor pipeline.  Stores batch 4 tiles per DMA.
            TAP_ORDER = (4, 0, 1, 2, 3, 5, 6, 7, 8)

            def tap_views(r, idx, dst):
                # (out_view, in_view) for tap idx of tile r; side taps write
                # only their in-row body (even sizes/offsets for fp32r
                # matmul ISA rules); the 2 edge columns per side are fixed
                # by batched per-group DVE ops below.
                i, j = idx // 3, idx % 3
                rows = slice(2 * r + i, 2 * r + i + 2)
                if j == 1:
                    return [(dst[:, :, :], x_sb[:, rows, 0:256])]
                if j == 0:
                    return [(dst[:, :, 2:256], x_sb[:, rows, 1:255])]
                return [(dst[:, :, 0:254], x_sb[:, rows, 1:255])]

            for g in range(NTILES // 4):
                st = sp.tile([128, 8, 256], F32, tag="st")
                n_dve = (0, 2, 1, 2)[g % 4]
                for sub in range(4):
                    r = 4 * g + sub
                    view = st[:, 2 * sub:2 * sub + 2, :]
                    if sub >= 4 - n_dve:
                        # center tap + const on ScalarE; 8 FMA taps on DVE
                        nc.scalar.activation(
                            view, x_sb[:, 2 * r + 1:2 * r + 3, :].bitcast(F32),
                            AF.Identity, bias=cvec[:], scale=wmat[:, 4:5],
                        )
                        for idx in TAP_ORDER[1:]:
                            for ov, iv in tap_views(r, idx, view):
                                nc.vector.scalar_tensor_tensor(
                                    ov, iv.bitcast(F32), wmat[:, idx:idx + 1],
                                    ov, ALU.mult, ALU.add,
                                )
                    else:
                        ps = pp.tile([128, 2, 256], F32, tag="ps")
                        n_mm = sum(len(tap_views(r, t, ps)) for t in TAP_ORDER)
                        mm = 0
                        for idx in TAP_ORDER:
                            for ov, iv in tap_views(r, idx, ps):
                                nc.tensor.matmul(
                                    ps[:, :, :] if ov is None else ov, diags[idx][:], iv,
                                    start=(mm == 0), stop=(mm == n_mm - 1),
                                )
                                mm += 1
                        nc.scalar.activation(view, ps[:], AF.Identity,
                                             bias=cvec[:])
                for i in range(3):
                    rows = slice(8 * g + i, 8 * g + i + 8)
                    nc.vector.scalar_tensor_tensor(
                        st[:, :, 0:2], x_sb[:, rows, 1::-1].bitcast(F32),
                        wmat[:, 3 * i:3 * i + 1], st[:, :, 0:2],
                        ALU.mult, ALU.add,
                    )
                    nc.vector.scalar_tensor_tensor(
                        st[:, :, 254:256],
                        x_sb[:, rows, 255:253:-1].bitcast(F32),
                        wmat[:, 3 * i + 2:3 * i + 3], st[:, :, 254:256],
                        ALU.mult, ALU.add,
                    )
                nc.sync.dma_start(out=out_d[:, 8 * g:8 * g + 8, :], in_=st[0:64])
                nc.sync.dma_start(
                    out=out_d[:, 128 + 8 * g:128 + 8 * g + 8, :], in_=st[64:128])

    nc.compile()
    return nc


def _host_consts(conv_w, bn_gamma, bn_beta, bn_mean, bn_var, lamb_l, lamb_h,
                 inside_all):
    f = np.float32
    eps = 1e-5
    bns = (bn_gamma / np.sqrt(bn_var + eps)).astype(f)          # (72,)
    bnb = (bn_beta - bn_mean * bns).astype(f)
    g = np.arange(128) % 64 // 8                                 # group of p
    g72 = np.zeros((72, 128), f)
    g72[np.arange(72)[:, None] // 9 == g[None, :]] = 0.0         # placeholder
    for p in range(128):
        for k in range(72):
            if k // 9 == g[p]:
                g72[k, p] = 1.0
    mask9 = np.zeros((72, K2), f)
    mask9[np.arange(72), np.arange(72) % 9] = 1.0
    ppool = np.zeros((128, 128), f)
    for m in range(128):
        ppool[m % 64, m] = 1.0 / 65536.0
        ppool[64 + m % 64, m] = 1.0 / 65536.0
    ia = inside_all.reshape(-1).astype(f)                        # (64,)
    ll = lamb_l.astype(f)
    lh = lamb_h.astype(f)
    a64 = (ll * (1.0 + ia)).astype(f)
    b64 = (1.0 + lh).astype(f)
    cl64 = (-ia * ll).astype(f)
    dup = lambda v: np.concatenate([v, v]).reshape(128, 1).astype(f)
    return dict(
        cwT=np.ascontiguousarray(conv_w.T.astype(f)),
        bns=bns.reshape(72, 1),
        bnb=bnb.reshape(72, 1),
        g72=g72,
        mask9=mask9,
        ppool=ppool,
        i128=np.eye(128, dtype=f),
        avec=dup(a64),
        bvec=dup(b64),
        clvec=dup(cl64),
    )


def kernel(x, conv_w, bn_gamma, bn_beta, bn_mean, bn_var, lamb_l, lamb_h,
           inside_all):
    x = np.asarray(x, np.float32)
    consts = _host_consts(
        np.asarray(conv_w, np.float32), np.asarray(bn_gamma, np.float32),
        np.asarray(bn_beta, np.float32), np.asarray(bn_mean, np.float32),
        np.asarray(bn_var, np.float32), np.asarray(lamb_l, np.float32),
        np.asarray(lamb_h, np.float32), np.asarray(inside_all, np.float32),
    )
    nc = _build_program()
    in_maps = [
        dict(x=np.ascontiguousarray(x[i]), **consts) for i in range(NCORES)
    ]
    trace = bool(os.environ.get("BASS_TRACE_KERNEL"))
    if trace:
        _install_ntff_hook()
    res = run_bass_kernel_spmd(
        nc, in_maps, core_ids=list(range(NCORES)), trace=trace
    )
    LAST_RESULT["exec_time_ns"] = res.exec_time_ns
    LAST_RESULT["raw"] = res
    return np.stack([res.results[i]["out"] for i in range(NCORES)], axis=0)



# revision 8
# speedup vs baseline: 1.1029x; 1.1029x over previous
"""Trainium2 Bass kernel for nn_DilatedAttention (dynamic per-image 3x3
depthwise filter + affine epilogue), data-parallel over batch on 8 cores.

Math per image (one core):
  pooled[c] = mean_hw(x)                              (64,)
  lf = tanh(BN(pooled @ conv_w.T))                    (72,) = (G=8, k2=9)
  low[c,h,w] = sum_t lf[g(c),t] * x[c, h+di, w+dj]    3x3 reflect-pad conv
  out = A[c]*low + B[c]*x + const[c]
    A = lamb_l*(1+inside_all), B = 1+lamb_h, const = -inside_all*lamb_l*pooled

v2: bf16 end-to-end.  x and out live in HBM as bf16 (host converts), which
halves DMA traffic; loads split across both HWDGE rings (SP + ACT); the
9-tap conv is spread across PE (diagonal bf16 matmuls into PSUM), DVE
(bf16 FMA chains), and GpSimd, with ScalarE doing PSUM evacuation and
center-tap passes.  Side taps use 255-wide views so only one column per
side needs a reflect fix (done on GpSimd).
"""

import os
import sys

import numpy as np

for _p in ("/opt/trn_rl_repo",):
    if _p not in sys.path:
        sys.path.insert(0, _p)

import concourse.bass as bass
import concourse.bacc as bacc
import concourse.mybir as mybir
import concourse.tile as tile
from concourse.bass_utils import run_bass_kernel_spmd

F32 = mybir.dt.float32
BF16 = mybir.dt.bfloat16
AF = mybir.ActivationFunctionType
ALU = mybir.AluOpType

C, H, W = 64, 256, 256
NCORES = 8
K2 = 9

# group schedule: 16 groups of 8 layout rows.  A = 4 PE tiles (2 rows each),
# B = 2 PE tiles + 1 DVE tile (4 rows).
GROUP_SCHED = list("BABBBBBBABBBBBBB")

LAST_RESULT = {}


def _install_ntff_hook():
    """Register the axon NTFF profile hook (the image's antenv lacks
    axon_hooks; build it from trn_agent_boot's ctypes shim)."""
    import types

    try:
        from antenv.axon_hooks import get_axon_ntff_profile_hook  # noqa: F401
        return
    except ImportError:
        pass
    mod = types.ModuleType("antenv.axon_hooks")
    _h = [None]
    mod.set_axon_ntff_profile_hook = lambda hook: _h.__setitem__(0, hook)
    mod.get_axon_ntff_profile_hook = lambda: _h[0]
    sys.modules["antenv.axon_hooks"] = mod
    import antenv

    antenv.axon_hooks = mod
    try:
        from trn_agent_boot.trn_boot import _ntff_profile_via_ctypes

        mod.set_axon_ntff_profile_hook(
            _ntff_profile_via_ctypes("/opt/axon/libaxon_pjrt.so")
        )
    except Exception as e:  # hook stays None; tracing degrades gracefully
        print("ntff hook install failed:", e)


def _build_program():
    nc = bacc.Bacc("TRN2", target_bir_lowering=False, debug=False)

    x_d = nc.declare_dram_parameter("x", [C, H, W], BF16, isOutput=False)
    out_d = nc.declare_dram_parameter("out", [C, H, W], BF16, isOutput=True)
    cwT_d = nc.declare_dram_parameter("cwT", [64, 72], F32, isOutput=False)
    bns_d = nc.declare_dram_parameter("bns", [72, 1], F32, isOutput=False)
    bnb_d = nc.declare_dram_parameter("bnb", [72, 1], F32, isOutput=False)
    g72_d = nc.declare_dram_parameter("g72", [72, 128], F32, isOutput=False)
    mask9_d = nc.declare_dram_parameter("mask9", [72, K2], F32, isOutput=False)
    ppool_d = nc.declare_dram_parameter("ppool", [128, 128], F32, isOutput=False)
    i128_d = nc.declare_dram_parameter("i128", [128, 128], BF16, isOutput=False)
    avec_d = nc.declare_dram_parameter("avec", [128, 1], F32, isOutput=False)
    bvec_d = nc.declare_dram_parameter("bvec", [128, 1], F32, isOutput=False)
    clvec_d = nc.declare_dram_parameter("clvec", [128, 1], F32, isOutput=False)

    with tile.TileContext(nc) as tc:
        with (
            tc.tile_pool(name="xbuf", bufs=1) as xp,
            tc.tile_pool(name="consts", bufs=1) as cp,
            tc.tile_pool(name="diag", bufs=1) as dp,
            tc.tile_pool(name="psum", bufs=5, space=bass.MemorySpace.PSUM) as pp,
            tc.tile_pool(name="stage", bufs=3) as sp,
            tc.tile_pool(name="spsum", bufs=1, space=bass.MemorySpace.PSUM) as pps,
        ):
            # ---- load x into SBUF (130 rows x 258 cols with reflect halo) ----
            # Layout col q holds image col q-1 (cols 0 and 257 are reflect
            # halos, filled by on-chip copies), so every 3x3 tap (i, j) is the
            # pure view x_sb[:, a+i:b+i, j:j+256] with no edge fixups.
            # Top half p<64: layout row r holds HBM row r-1; bottom half
            # p>=64: layout r holds HBM row 127+r.  Top half loads through the
            # SP HWDGE ring, bottom half through the ACT ring, so the two
            # 64-partition streams use independent queues.  6 chunks + 2-row
            # gaps per half let pooling overlap the load.
            x_sb = xp.tile([128, 130, 258], BF16)
            segs = [(1, 22), (24, 45), (47, 68), (70, 91), (93, 114), (116, 129)]
            gaps = [(22, 24), (45, 47), (68, 70), (91, 93), (114, 116)]
            for a, b in segs + gaps:
                nc.sync.dma_start(out=x_sb[0:64, a:b, 1:257],
                                  in_=x_d[:, a - 1:b - 1, :])
                nc.scalar.dma_start(out=x_sb[64:128, a:b, 1:257],
                                    in_=x_d[:, 127 + a:127 + b, :])
            # top halo-bottom (HBM row 128) and bottom halo-top (HBM row 127)
            nc.sync.dma_start(out=x_sb[0:64, 129:130, 1:257],
                              in_=x_d[:, 128:129, :])
            nc.scalar.dma_start(out=x_sb[64:128, 0:1, 1:257],
                                in_=x_d[:, 127:128, :])
            # reflect halo rows and columns via on-chip copies
            nc.vector.tensor_copy(x_sb[0:64, 0:1, 1:257], x_sb[0:64, 2:3, 1:257])
            nc.vector.tensor_copy(x_sb[64:128, 129:130, 1:257],
                                  x_sb[64:128, 127:128, 1:257])
            nc.vector.tensor_copy(x_sb[:, :, 0:1], x_sb[:, :, 2:3])
            nc.vector.tensor_copy(x_sb[:, :, 257:258], x_sb[:, :, 255:256])

            # ---- small constants ----
            cwT = cp.tile([64, 72], F32, tag="cwT")
            bns = cp.tile([72, 1], F32, tag="bns")
            bnb = cp.tile([72, 1], F32, tag="bnb")
            g72 = cp.tile([72, 128], F32, tag="g72")
            mask9 = cp.tile([72, K2], F32, tag="mask9")
            ppool = cp.tile([128, 128], F32, tag="ppool")
            i128 = cp.tile([128, 128], BF16, tag="i128")
            avec = cp.tile([128, 1], F32, tag="avec")
            bvec = cp.tile([128, 1], F32, tag="bvec")
            clvec = cp.tile([128, 1], F32, tag="clvec")
            for t, d in (
                (cwT, cwT_d), (bns, bns_d), (bnb, bnb_d), (g72, g72_d),
                (mask9, mask9_d), (ppool, ppool_d), (i128, i128_d),
                (avec, avec_d), (bvec, bvec_d), (clvec, clvec_d),
            ):
                nc.sync.dma_start(out=t[:], in_=d[:])

            # ---- pooling over layout rows 1..128, chunk-aligned w/ DMAs ----
            # All reduces on DVE (bf16 2x), both halves per chunk; runs under
            # the load.
            chunks = segs + gaps
            NPS = len(chunks)
            pstat = cp.tile([128, NPS], F32, tag="pstat")
            for k, (a, b) in enumerate(chunks):
                nc.vector.tensor_reduce(
                    out=pstat[:, k:k + 1],
                    in_=x_sb[:, a:b, 1:257],
                    axis=mybir.AxisListType.XY,
                    op=ALU.add,
                )
            stat = cp.tile([128, 1], F32, tag="stat")
            nc.vector.tensor_reduce(
                out=stat[:], in_=pstat[:], axis=mybir.AxisListType.X, op=ALU.add
            )

            # pooled[p] = (stat[p%64] + stat[64+p%64]) / 65536  (both halves)
            pooled_ps = pps.tile([128, 1], F32, tag="pooled_ps")
            nc.tensor.matmul(pooled_ps[:], ppool[:], stat[:])
            pooled = cp.tile([128, 1], F32, tag="pooled")
            nc.scalar.copy(pooled[:], pooled_ps[:])

            # const[p] = CL[p] * pooled[p]
            cvec = cp.tile([128, 1], F32, tag="cvec")
            nc.vector.tensor_scalar_mul(cvec[:], pooled[:], clvec[:])

            # lf = tanh(bns * (pooled @ conv_w.T) + bnb)   [72,1]
            lf_ps = pps.tile([72, 1], F32, tag="lf_ps")
            nc.tensor.matmul(lf_ps[:], cwT[:], pooled[0:64, :])
            lf = cp.tile([72, 1], F32, tag="lf")
            nc.scalar.activation(lf[:], lf_ps[:], AF.Tanh, bias=bnb[:], scale=bns[:])

            # W0[p,t] = lf[g(p)*9+t]:  lfmat = mask9 * lf ; W0 = g72.T @ lfmat
            lfmat = cp.tile([72, K2], F32, tag="lfmat")
            nc.vector.tensor_scalar_mul(lfmat[:], mask9[:], lf[:])
            w_ps = pps.tile([128, K2], F32, tag="w_ps")
            nc.tensor.matmul(w_ps[:], g72[:], lfmat[:])
            # W = A * W0 ; then center tap += B  (folds B*x into the conv)
            wmat = cp.tile([128, K2], F32, tag="wmat")
            nc.scalar.activation(wmat[:], w_ps[:], AF.Copy, scale=avec[:])
            nc.vector.tensor_scalar_add(wmat[:, 4:5], wmat[:, 4:5], bvec[:])

            # diagonal stationary matrices D_t = diag(W[:,t]) in bf16
            diags = []
            for t in range(K2):
                d_t = dp.tile([128, 128], BF16, tag=f"d{t}")
                nc.vector.tensor_scalar_mul(d_t[:], i128[:], wmat[:, t:t + 1])
                diags.append(d_t)

            # ---- main loop ----
            # Output tile covering layout rows [a, b): tap (i, j) is the pure
            # view x_sb[:, a+i:b+i, j:j+256] thanks to the halo columns.
            TAP_ORDER = (4, 0, 1, 2, 3, 5, 6, 7, 8)

            def tap_in(idx, a, b):
                i, j = idx // 3, idx % 3
                return x_sb[:, a + i:b + i, j:j + 256]

            def fma_tile(eng, view, a, b):
                # center tap + const on ScalarE, 8 FMA taps on eng
                nc.scalar.activation(
                    view, tap_in(4, a, b),
                    AF.Identity, bias=cvec[:], scale=wmat[:, 4:5],
                )
                for idx in TAP_ORDER[1:]:
                    eng.scalar_tensor_tensor(
                        view, tap_in(idx, a, b), wmat[:, idx:idx + 1],
                        view, ALU.mult, ALU.add,
                    )

            def pe_tiles(st, offs, g):
                # 2-row PE tiles at st row offsets offs; tap-major so the
                # same stationary is reused across consecutive matmuls
                pss = []
                for o in offs:
                    pss.append((pp.tile([128, 2, 256], F32, tag="ps",
                                        name=f"ps{g}_{o}"), o))
                for ti, idx in enumerate(TAP_ORDER):
                    for ps, o in pss:
                        a = 8 * g + o
                        nc.tensor.matmul(
                            ps[:], diags[idx][:], tap_in(idx, a, a + 2),
                            start=(ti == 0), stop=(ti == len(TAP_ORDER) - 1),
                        )
                for ps, o in pss:
                    nc.scalar.activation(
                        st[:, o:o + 2, :], ps[:], AF.Identity, bias=cvec[:]
                    )

            for g, typ in enumerate(GROUP_SCHED):
                st = sp.tile([128, 8, 256], BF16, tag="st")
                if typ == "A":
                    pe_tiles(st, (0, 2, 4, 6), g)
                else:  # B
                    pe_tiles(st, (0, 2), g)
                    fma_tile(nc.vector, st[:, 4:8, :], 8 * g + 4, 8 * g + 8)
                nc.sync.dma_start(out=out_d[:, 8 * g:8 * g + 8, :], in_=st[0:64])
                nc.gpsimd.dma_start(
                    out=out_d[:, 128 + 8 * g:128 + 8 * g + 8, :], in_=st[64:128])

    nc.compile()
    return nc


def _host_consts(conv_w, bn_gamma, bn_beta, bn_mean, bn_var, lamb_l, lamb_h,
                 inside_all):
    import ml_dtypes

    f = np.float32
    eps = 1e-5
    bns = (bn_gamma / np.sqrt(bn_var + eps)).astype(f)          # (72,)
    bnb = (bn_beta - bn_mean * bns).astype(f)
    g = np.arange(128) % 64 // 8                                 # group of p
    g72 = np.zeros((72, 128), f)
    for p in range(128):
        for k in range(72):
            if k // 9 == g[p]:
                g72[k, p] = 1.0
    mask9 = np.zeros((72, K2), f)
    mask9[np.arange(72), np.arange(72) % 9] = 1.0
    ppool = np.zeros((128, 128), f)
    for m in range(128):
        ppool[m % 64, m] = 1.0 / 65536.0
        ppool[64 + m % 64, m] = 1.0 / 65536.0
    ia = inside_all.reshape(-1).astype(f)                        # (64,)
    ll = lamb_l.astype(f)
    lh = lamb_h.astype(f)
    a64 = (ll * (1.0 + ia)).astype(f)
    b64 = (1.0 + lh).astype(f)
    cl64 = (-ia * ll).astype(f)
    dup = lambda v: np.concatenate([v, v]).reshape(128, 1).astype(f)
    return dict(
        cwT=np.ascontiguousarray(conv_w.T.astype(f)),
        bns=bns.reshape(72, 1),
        bnb=bnb.reshape(72, 1),
        g72=g72,
        mask9=mask9,
        ppool=ppool,
        i128=np.eye(128, dtype=ml_dtypes.bfloat16),
        avec=dup(a64),
        bvec=dup(b64),
        clvec=dup(cl64),
    )


def kernel(x, conv_w, bn_gamma, bn_beta, bn_mean, bn_var, lamb_l, lamb_h,
           inside_all):
    import ml_dtypes

    x = np.asarray(x, np.float32).astype(ml_dtypes.bfloat16)
    consts = _host_consts(
        np.asarray(conv_w, np.float32), np.asarray(bn_gamma, np.float32),
        np.asarray(bn_beta, np.float32), np.asarray(bn_mean, np.float32),
        np.asarray(bn_var, np.float32), np.asarray(lamb_l, np.float32),
        np.asarray(lamb_h, np.float32), np.asarray(inside_all, np.float32),
    )
    nc = _build_program()
    in_maps = [
        dict(x=np.ascontiguousarray(x[i]), **consts) for i in range(NCORES)
    ]
    trace = bool(os.environ.get("BASS_TRACE_KERNEL"))
    if trace:
        _install_ntff_hook()
    res = run_bass_kernel_spmd(
        nc, in_maps, core_ids=list(range(NCORES)), trace=trace
    )
    LAST_RESULT["exec_time_ns"] = res.exec_time_ns
    LAST_RESULT["raw"] = res
    return np.stack(
        [res.results[i]["out"].astype(np.float32) for i in range(NCORES)], axis=0
    )


# revision 11
# speedup vs baseline: 1.3147x; 1.1920x over previous
"""Trainium2 Bass kernel for nn_DilatedAttention (dynamic per-image 3x3
depthwise filter + affine epilogue), data-parallel over batch on 8 cores.

Math per image (one core):
  pooled[c] = mean_hw(x)                              (64,)
  lf = tanh(BN(pooled @ conv_w.T))                    (72,) = (G=8, k2=9)
  low[c,h,w] = sum_t lf[g(c),t] * x[c, h+di, w+dj]    3x3 reflect-pad conv
  out = A[c]*low + B[c]*x + const[c]
    A = lamb_l*(1+inside_all), B = 1+lamb_h, const = -inside_all*lamb_l*pooled

v2: bf16 end-to-end.  x and out live in HBM as bf16 (host converts), which
halves DMA traffic; loads split across both HWDGE rings (SP + ACT); the
9-tap conv is spread across PE (diagonal bf16 matmuls into PSUM), DVE
(bf16 FMA chains), and GpSimd, with ScalarE doing PSUM evacuation and
center-tap passes.  Side taps use 255-wide views so only one column per
side needs a reflect fix (done on GpSimd).
"""

import os
import sys

import numpy as np

for _p in ("/opt/trn_rl_repo",):
    if _p not in sys.path:
        sys.path.insert(0, _p)

import concourse.bass as bass
import concourse.bacc as bacc
import concourse.mybir as mybir
import concourse.tile as tile
from concourse.bass_utils import run_bass_kernel_spmd

F32 = mybir.dt.float32
BF16 = mybir.dt.bfloat16
AF = mybir.ActivationFunctionType
ALU = mybir.AluOpType

C, H, W = 64, 256, 256
NCORES = 8
K2 = 9

# group schedule: 16 groups of 8 layout rows.  A = 4 PE tiles (2 rows each),
# B = 2 PE tiles + 1 DVE tile (4 rows).  PE sustains ~216 ns per 512-row
# bf16 matmul once warm, so it carries ~92 of the 128 rows.
GROUP_SCHED = list("ABBABABBABABBABA")

LAST_RESULT = {}


def _install_ntff_hook():
    """Register the axon NTFF profile hook (the image's antenv lacks
    axon_hooks; build it from trn_agent_boot's ctypes shim)."""
    import types

    try:
        from antenv.axon_hooks import get_axon_ntff_profile_hook  # noqa: F401
        return
    except ImportError:
        pass
    mod = types.ModuleType("antenv.axon_hooks")
    _h = [None]
    mod.set_axon_ntff_profile_hook = lambda hook: _h.__setitem__(0, hook)
    mod.get_axon_ntff_profile_hook = lambda: _h[0]
    sys.modules["antenv.axon_hooks"] = mod
    import antenv

    antenv.axon_hooks = mod
    try:
        from trn_agent_boot.trn_boot import _ntff_profile_via_ctypes

        mod.set_axon_ntff_profile_hook(
            _ntff_profile_via_ctypes("/opt/axon/libaxon_pjrt.so")
        )
    except Exception as e:  # hook stays None; tracing degrades gracefully
        print("ntff hook install failed:", e)


def _build_program():
    nc = bacc.Bacc("TRN2", target_bir_lowering=False, debug=False)

    x_d = nc.declare_dram_parameter("x", [C, H, W], BF16, isOutput=False)
    out_d = nc.declare_dram_parameter("out", [C, H, W], BF16, isOutput=True)
    cwT_d = nc.declare_dram_parameter("cwT", [64, 72], F32, isOutput=False)
    bns_d = nc.declare_dram_parameter("bns", [72, 1], F32, isOutput=False)
    bnb_d = nc.declare_dram_parameter("bnb", [72, 1], F32, isOutput=False)
    g72_d = nc.declare_dram_parameter("g72", [72, 128], F32, isOutput=False)
    mask9_d = nc.declare_dram_parameter("mask9", [72, K2], F32, isOutput=False)
    ppool_d = nc.declare_dram_parameter("ppool", [128, 128], F32, isOutput=False)
    i128_d = nc.declare_dram_parameter("i128", [128, 128], BF16, isOutput=False)
    avec_d = nc.declare_dram_parameter("avec", [128, 1], F32, isOutput=False)
    bvec_d = nc.declare_dram_parameter("bvec", [128, 1], F32, isOutput=False)
    clvec_d = nc.declare_dram_parameter("clvec", [128, 1], F32, isOutput=False)

    with tile.TileContext(nc) as tc:
        with (
            tc.tile_pool(name="xbuf", bufs=1) as xp,
            tc.tile_pool(name="consts", bufs=1) as cp,
            tc.tile_pool(name="diag", bufs=1) as dp,
            tc.tile_pool(name="psum", bufs=5, space=bass.MemorySpace.PSUM) as pp,
            tc.tile_pool(name="stage", bufs=3) as sp,
            tc.tile_pool(name="spsum", bufs=1, space=bass.MemorySpace.PSUM) as pps,
        ):
            # ---- load x into SBUF (130 rows x 258 cols with reflect halo) ----
            # Layout col q holds image col q-1 (cols 0 and 257 are reflect
            # halos, filled by on-chip copies), so every 3x3 tap (i, j) is the
            # pure view x_sb[:, a+i:b+i, j:j+256] with no edge fixups.
            # Top half p<64: layout row r holds HBM row r-1; bottom half
            # p>=64: layout r holds HBM row 127+r.  Top half loads through the
            # SP HWDGE ring, bottom half through the ACT ring, so the two
            # 64-partition streams use independent queues.  6 chunks + 2-row
            # gaps per half let pooling overlap the load.
            x_sb = xp.tile([128, 130, 258], BF16)
            # few, big DMAs (per-dma fixed cost ~2us dominates small ones);
            # a small final chunk keeps the pooling tail short
            segs = [(1, 50), (50, 99), (99, 120), (120, 129)]
            gaps = []
            for a, b in segs + gaps:
                nc.sync.dma_start(out=x_sb[0:64, a:b, 1:257],
                                  in_=x_d[:, a - 1:b - 1, :])
                nc.scalar.dma_start(out=x_sb[64:128, a:b, 1:257],
                                    in_=x_d[:, 127 + a:127 + b, :])
            # top halo-bottom (HBM row 128) and bottom halo-top (HBM row 127)
            nc.sync.dma_start(out=x_sb[0:64, 129:130, 1:257],
                              in_=x_d[:, 128:129, :])
            nc.scalar.dma_start(out=x_sb[64:128, 0:1, 1:257],
                                in_=x_d[:, 127:128, :])
            # reflect halo rows and columns via on-chip copies
            nc.vector.tensor_copy(x_sb[0:64, 0:1, 1:257], x_sb[0:64, 2:3, 1:257])
            nc.vector.tensor_copy(x_sb[64:128, 129:130, 1:257],
                                  x_sb[64:128, 127:128, 1:257])
            nc.vector.tensor_copy(x_sb[:, :, 0:1], x_sb[:, :, 2:3])
            nc.vector.tensor_copy(x_sb[:, :, 257:258], x_sb[:, :, 255:256])

            # ---- small constants ----
            cwT = cp.tile([64, 72], F32, tag="cwT")
            bns = cp.tile([72, 1], F32, tag="bns")
            bnb = cp.tile([72, 1], F32, tag="bnb")
            g72 = cp.tile([72, 128], F32, tag="g72")
            mask9 = cp.tile([72, K2], F32, tag="mask9")
            ppool = cp.tile([128, 128], F32, tag="ppool")
            i128 = cp.tile([128, 128], BF16, tag="i128")
            avec = cp.tile([128, 1], F32, tag="avec")
            bvec = cp.tile([128, 1], F32, tag="bvec")
            clvec = cp.tile([128, 1], F32, tag="clvec")
            for t, d in (
                (cwT, cwT_d), (bns, bns_d), (bnb, bnb_d), (g72, g72_d),
                (mask9, mask9_d), (ppool, ppool_d), (i128, i128_d),
                (avec, avec_d), (bvec, bvec_d), (clvec, clvec_d),
            ):
                nc.sync.dma_start(out=t[:], in_=d[:])

            # ---- pooling over layout rows 1..128, chunk-aligned w/ DMAs ----
            # All reduces on DVE (bf16 2x), both halves per chunk; runs under
            # the load.
            chunks = segs + gaps
            NPS = len(chunks)
            pstat = cp.tile([128, NPS], F32, tag="pstat")
            for k, (a, b) in enumerate(chunks):
                nc.vector.tensor_reduce(
                    out=pstat[:, k:k + 1],
                    in_=x_sb[:, a:b, 1:257],
                    axis=mybir.AxisListType.XY,
                    op=ALU.add,
                )
            stat = cp.tile([128, 1], F32, tag="stat")
            nc.vector.tensor_reduce(
                out=stat[:], in_=pstat[:], axis=mybir.AxisListType.X, op=ALU.add
            )

            # pooled[p] = (stat[p%64] + stat[64+p%64]) / 65536  (both halves)
            pooled_ps = pps.tile([128, 1], F32, tag="pooled_ps")
            nc.tensor.matmul(pooled_ps[:], ppool[:], stat[:])
            pooled = cp.tile([128, 1], F32, tag="pooled")
            nc.scalar.copy(pooled[:], pooled_ps[:])

            # const[p] = CL[p] * pooled[p]
            cvec = cp.tile([128, 1], F32, tag="cvec")
            nc.vector.tensor_scalar_mul(cvec[:], pooled[:], clvec[:])

            # lf = tanh(bns * (pooled @ conv_w.T) + bnb)   [72,1]
            lf_ps = pps.tile([72, 1], F32, tag="lf_ps")
            nc.tensor.matmul(lf_ps[:], cwT[:], pooled[0:64, :])
            lf = cp.tile([72, 1], F32, tag="lf")
            nc.scalar.activation(lf[:], lf_ps[:], AF.Tanh, bias=bnb[:], scale=bns[:])

            # W0[p,t] = lf[g(p)*9+t]:  lfmat = mask9 * lf ; W0 = g72.T @ lfmat
            lfmat = cp.tile([72, K2], F32, tag="lfmat")
            nc.vector.tensor_scalar_mul(lfmat[:], mask9[:], lf[:])
            w_ps = pps.tile([128, K2], F32, tag="w_ps")
            nc.tensor.matmul(w_ps[:], g72[:], lfmat[:])
            # W = A * W0 ; then center tap += B  (folds B*x into the conv)
            wmat = cp.tile([128, K2], F32, tag="wmat")
            nc.scalar.activation(wmat[:], w_ps[:], AF.Copy, scale=avec[:])
            nc.vector.tensor_scalar_add(wmat[:, 4:5], wmat[:, 4:5], bvec[:])

            # diagonal stationary matrices D_t = diag(W[:,t]) in bf16
            diags = []
            for t in range(K2):
                d_t = dp.tile([128, 128], BF16, tag=f"d{t}")
                nc.vector.tensor_scalar_mul(d_t[:], i128[:], wmat[:, t:t + 1])
                diags.append(d_t)

            # ---- main loop ----
            # Output tile covering layout rows [a, b): tap (i, j) is the pure
            # view x_sb[:, a+i:b+i, j:j+256] thanks to the halo columns.
            TAP_ORDER = (4, 0, 1, 2, 3, 5, 6, 7, 8)

            def tap_in(idx, a, b):
                i, j = idx // 3, idx % 3
                return x_sb[:, a + i:b + i, j:j + 256]

            def fma_tile(eng, view, a, b):
                # center tap + const on ScalarE, 8 FMA taps on eng
                nc.scalar.activation(
                    view, tap_in(4, a, b),
                    AF.Identity, bias=cvec[:], scale=wmat[:, 4:5],
                )
                for idx in TAP_ORDER[1:]:
                    eng.scalar_tensor_tensor(
                        view, tap_in(idx, a, b), wmat[:, idx:idx + 1],
                        view, ALU.mult, ALU.add,
                    )

            def pe_tiles(st, offs, g):
                # 2-row PE tiles at st row offsets offs; tap-major so the
                # same stationary is reused across consecutive matmuls
                pss = []
                for o in offs:
                    pss.append((pp.tile([128, 2, 256], F32, tag="ps",
                                        name=f"ps{g}_{o}"), o))
                for ti, idx in enumerate(TAP_ORDER):
                    for ps, o in pss:
                        a = 8 * g + o
                        nc.tensor.matmul(
                            ps[:], diags[idx][:], tap_in(idx, a, a + 2),
                            start=(ti == 0), stop=(ti == len(TAP_ORDER) - 1),
                        )
                for ps, o in pss:
                    nc.scalar.activation(
                        st[:, o:o + 2, :], ps[:], AF.Identity, bias=cvec[:]
                    )

            for g, typ in enumerate(GROUP_SCHED):
                st = sp.tile([128, 8, 256], BF16, tag="st")
                if typ == "A":
                    pe_tiles(st, (0, 2, 4, 6), g)
                else:  # B
                    pe_tiles(st, (0, 2), g)
                    fma_tile(nc.vector, st[:, 4:8, :], 8 * g + 4, 8 * g + 8)
                nc.sync.dma_start(out=out_d[:, 8 * g:8 * g + 8, :], in_=st[0:64])
                nc.sync.dma_start(
                    out=out_d[:, 128 + 8 * g:128 + 8 * g + 8, :], in_=st[64:128])

    nc.compile()
    return nc


def _host_consts(conv_w, bn_gamma, bn_beta, bn_mean, bn_var, lamb_l, lamb_h,
                 inside_all):
    import ml_dtypes

    f = np.float32
    eps = 1e-5
    bns = (bn_gamma / np.sqrt(bn_var + eps)).astype(f)          # (72,)
    bnb = (bn_beta - bn_mean * bns).astype(f)
    g = np.arange(128) % 64 // 8                                 # group of p
    g72 = np.zeros((72, 128), f)
    for p in range(128):
        for k in range(72):
            if k // 9 == g[p]:
                g72[k, p] = 1.0
    mask9 = np.zeros((72, K2), f)
    mask9[np.arange(72), np.arange(72) % 9] = 1.0
    ppool = np.zeros((128, 128), f)
    for m in range(128):
        ppool[m % 64, m] = 1.0 / 65536.0
        ppool[64 + m % 64, m] = 1.0 / 65536.0
    ia = inside_all.reshape(-1).astype(f)                        # (64,)
    ll = lamb_l.astype(f)
    lh = lamb_h.astype(f)
    a64 = (ll * (1.0 + ia)).astype(f)
    b64 = (1.0 + lh).astype(f)
    cl64 = (-ia * ll).astype(f)
    dup = lambda v: np.concatenate([v, v]).reshape(128, 1).astype(f)
    return dict(
        cwT=np.ascontiguousarray(conv_w.T.astype(f)),
        bns=bns.reshape(72, 1),
        bnb=bnb.reshape(72, 1),
        g72=g72,
        mask9=mask9,
        ppool=ppool,
        i128=np.eye(128, dtype=ml_dtypes.bfloat16),
        avec=dup(a64),
        bvec=dup(b64),
        clvec=dup(cl64),
    )


def kernel(x, conv_w, bn_gamma, bn_beta, bn_mean, bn_var, lamb_l, lamb_h,
           inside_all):
    import ml_dtypes

    x = np.asarray(x, np.float32).astype(ml_dtypes.bfloat16)
    consts = _host_consts(
        np.asarray(conv_w, np.float32), np.asarray(bn_gamma, np.float32),
        np.asarray(bn_beta, np.float32), np.asarray(bn_mean, np.float32),
        np.asarray(bn_var, np.float32), np.asarray(lamb_l, np.float32),
        np.asarray(lamb_h, np.float32), np.asarray(inside_all, np.float32),
    )
    nc = _build_program()
    in_maps = [
        dict(x=np.ascontiguousarray(x[i]), **consts) for i in range(NCORES)
    ]
    trace = bool(os.environ.get("BASS_TRACE_KERNEL"))
    if trace:
        _install_ntff_hook()
    res = run_bass_kernel_spmd(
        nc, in_maps, core_ids=list(range(NCORES)), trace=trace
    )
    LAST_RESULT["exec_time_ns"] = res.exec_time_ns
    LAST_RESULT["raw"] = res
    return np.stack(
        [res.results[i]["out"].astype(np.float32) for i in range(NCORES)], axis=0
    )
